# revision 18
# baseline (speedup 1.0000x reference)
"""Trainium2 Bass kernel for nn_GAT_44487271252524.

GAT -> per-graph pairwise attention -> GAT, data-parallel over the 64 graphs
(8 graphs per NeuronCore), with the edge message-passing sharded by
destination node and everything else replicated. One AllGather moves the
first GAT's output (x1^T) between the sharded and replicated stages.
"""
import os
import numpy as np

import concourse.bass as bass
import concourse.bacc as bacc
import concourse.mybir as mybir
import concourse.tile as tile
from concourse.bass_utils import run_bass_kernel_spmd
from concourse.masks import make_identity

N_CORES = 8
B = 64
PAST = 128
FUTURE = 64
HID = 64
HEADS = 4
F = 51          # input feature dim
FO = F - 1      # GAT1 output dim (50)
N1 = B * PAST   # 8192 past nodes
N2 = B * FUTURE  # 4096 future nodes
GPC = B // N_CORES          # graphs per core (8)
T1_TILES = GPC * PAST // 128   # GAT1 dst tiles per core (8)
T2_TILES = GPC * FUTURE // 128  # GAT2 dst tiles per core (4)
ROW1 = 256      # f32 per T1 row (1024B): [h(200) | asrc(4) | pad]
ROW1D = 64      # f32 per T1d row (256B): [adst(4) | pad]
ROW2 = 64       # f32 per T2 row (256B): [h2(4) | asrc2(4) | adst2(4) | pad]
NEG = -1.0e30

_cache = {}


def _wrap_idx(idx, width):
    """int16 index array -> dma_gather layout [128, width/?]: idx i at
    [i%16, i//16], replicated across the 8 Q7 core groups."""
    n = idx.shape[0]
    assert n % 16 == 0
    out = np.zeros((128, n // 16), dtype=np.int16)
    w = idx.reshape(n // 16, 16).T
    for g in range(8):
        out[g * 16:(g + 1) * 16, :] = w
    return out


def _edge_prep(src, dst, n_dst_per_core, tile_count):
    """Split edges by core (dst) and 128-node dst tile. Returns
    (nchunk, per_core dict arrays) with every tile padded to nchunk*128."""
    order = np.argsort(dst, kind="stable")
    src = src[order]
    dst = dst[order]
    core_of = dst // n_dst_per_core
    tile_of = (dst % n_dst_per_core) // 128
    counts = np.zeros((N_CORES, tile_count), dtype=np.int64)
    np.add.at(counts, (core_of, tile_of), 1)
    nchunk = int(np.ceil(counts.max() / 128))
    epad = nchunk * 128
    srcs = np.zeros((N_CORES, tile_count, epad), dtype=np.int64)
    dsts = np.zeros((N_CORES, tile_count, epad), dtype=np.int64)
    dloc = np.full((N_CORES, tile_count, epad), -1.0, dtype=np.float32)
    key = core_of * tile_count + tile_of
    korder = np.argsort(key, kind="stable")
    src = src[korder]
    dst = dst[korder]
    key = key[korder]
    starts = np.searchsorted(key, np.arange(N_CORES * tile_count))
    ends = np.searchsorted(key, np.arange(N_CORES * tile_count), side="right")
    for c in range(N_CORES):
        for t in range(tile_count):
            a, b_ = starts[c * tile_count + t], ends[c * tile_count + t]
            k = b_ - a
            srcs[c, t, :k] = src[a:b_]
            dsts[c, t, :k] = dst[a:b_]
            dloc[c, t, :k] = (dst[a:b_] % 128).astype(np.float32)
    return nchunk, srcs, dsts, dloc


def _prep_inputs(cat1, num1, cat2, num2, e1, e2, A, emb0, emb1, emb2,
                 g1_lin, g1_asrc, g1_adst, g1_b, g2_lin, g2_asrc, g2_adst,
                 g2_b, W):
    f32 = np.float32
    cat1 = np.asarray(cat1).astype(np.int64)
    cat2 = np.asarray(cat2).astype(np.int64)
    e1 = np.asarray(e1).astype(np.int64)
    e2 = np.asarray(e2).astype(np.int64)

    # one-hot tables (emb2 compacted over observed values)
    uniq = np.unique(np.concatenate([cat1[:, 2], cat2[:, 2]]))
    assert uniq.shape[0] <= 128, "too many distinct emb2 indices"
    remap = np.zeros(int(uniq.max()) + 1, dtype=np.int64)
    remap[uniq] = np.arange(uniq.shape[0])
    emb2c = np.asarray(emb2, f32)[uniq]          # [U, 24]

    def onehot(vals, depth):
        oh = np.zeros((depth, vals.shape[0]), dtype=f32)
        oh[vals, np.arange(vals.shape[0])] = 1.0
        return oh

    U = uniq.shape[0]
    num1T = np.ascontiguousarray(np.asarray(num1, f32).T)   # [3, N1]
    num2T = np.ascontiguousarray(np.asarray(num2, f32).T)   # [3, N2]
    comb1 = np.concatenate([onehot(cat1[:, 0], 24), onehot(cat1[:, 1], 7),
                            onehot(remap[cat1[:, 2]], U), num1T])  # [34+U, N1]
    comb2 = np.concatenate([onehot(cat2[:, 0], 24), onehot(cat2[:, 1], 7),
                            onehot(remap[cat2[:, 2]], U), num2T])  # [34+U, N2]
    # block lhsT [34+U, 51]: one-hot-indexed rows -> feature columns
    emb_block = np.zeros((34 + U, F), dtype=f32)
    emb_block[0:24, 0:16] = np.asarray(emb0, f32)
    emb_block[24:31, 16:24] = np.asarray(emb1, f32)
    emb_block[31:31 + U, 24:48] = emb2c
    emb_block[31 + U:34 + U, 48:51] = np.eye(3, dtype=f32)

    g1_lin = np.asarray(g1_lin, f32)
    g1_asrc = np.asarray(g1_asrc, f32)
    g1_adst = np.asarray(g1_adst, f32)
    w1_asrc = np.stack([g1_lin[:, h * FO:(h + 1) * FO] @ g1_asrc[h]
                        for h in range(HEADS)], axis=1)     # [51, 4]
    w1_adst = np.stack([g1_lin[:, h * FO:(h + 1) * FO] @ g1_adst[h]
                        for h in range(HEADS)], axis=1)
    rhs1 = np.concatenate([g1_lin, w1_asrc, w1_adst], axis=1)  # [51, 208]

    g2_lin = np.asarray(g2_lin, f32)
    w2_asrc = g2_lin * np.asarray(g2_asrc, f32)[:, 0][None, :]  # [51, 4]
    w2_adst = g2_lin * np.asarray(g2_adst, f32)[:, 0][None, :]
    rhs2 = np.concatenate([g2_lin, w2_asrc, w2_adst], axis=1)   # [51, 12]

    W = np.asarray(W, f32)
    M = W @ W.T
    M = (M + M.T).astype(f32)                                   # [50, 50]

    maskA = np.where(np.asarray(A)[:PAST, PAST:].T == 0, f32(NEG), f32(0.0))
    mask_pair = np.full((128, 256), f32(NEG), dtype=f32)
    mask_pair[0:64, 0:128] = maskA
    mask_pair[64:128, 128:256] = maskA

    b1rep = np.tile(np.asarray(g1_b, f32)[None, :], (128, 1))   # [128, 50]
    b2 = float(np.asarray(g2_b, f32)[0])

    # edges + self loops
    loops1 = np.arange(N1, dtype=np.int64)
    s1 = np.concatenate([e1[0], loops1])
    d1 = np.concatenate([e1[1], loops1])
    loops2 = np.arange(N2, dtype=np.int64)
    s2 = np.concatenate([e2[0], loops2])
    d2 = np.concatenate([e2[1], loops2])

    nchunk1, srcs1, dsts1, dloc1 = _edge_prep(s1, d1, N1 // N_CORES, T1_TILES)
    nchunk2, srcs2, dsts2, dloc2 = _edge_prep(s2, d2, N2 // N_CORES, T2_TILES)

    shared = dict(
        comb1=comb1, comb2=comb2, emb_block=emb_block,
        rhs1=rhs1, rhs2=rhs2[0:FO].copy(), rhs2_tmp=rhs2[FO:F].copy(), m_mat=M,
        mask_pair=mask_pair, b1rep=b1rep,
    )
    per_core = []
    for c in range(N_CORES):
        d = dict(shared)
        d["src1w"] = np.stack([_wrap_idx(srcs1[c, t].astype(np.int16), 0)
                               for t in range(T1_TILES)])
        d["dst1w"] = np.stack([_wrap_idx(dsts1[c, t].astype(np.int16), 0)
                               for t in range(T1_TILES)])
        d["d1loc"] = np.stack([dloc1[c, t].reshape(nchunk1, 128).T.copy()
                               for t in range(T1_TILES)])
        d["src2w"] = np.stack([_wrap_idx(srcs2[c, t].astype(np.int16), 0)
                               for t in range(T2_TILES)])
        d["dst2w"] = np.stack([_wrap_idx(dsts2[c, t].astype(np.int16), 0)
                               for t in range(T2_TILES)])
        d["d2loc"] = np.stack([dloc2[c, t].reshape(nchunk2, 128).T.copy()
                               for t in range(T2_TILES)])
        per_core.append(d)
    return nchunk1, nchunk2, uniq.shape[0], b2, per_core


def _build(nchunk1, nchunk2, n_uniq, b2):
    stage = os.environ.get("K_STAGE", "F")
    csub = os.environ.get("K_CSUB", "full")
    no_cc = os.environ.get("K_NOCC", "0") == "1"
    f32 = mybir.dt.float32
    nc = bacc.Bacc("TRN2", target_bir_lowering=False, num_devices=N_CORES)
    E1P = nchunk1 * 128
    E2P = nchunk2 * 128
    W1 = E1P // 16
    W2 = E2P // 16

    def inp(name, shape, dtype=f32):
        return nc.dram_tensor(name, shape, dtype, kind="ExternalInput")

    KE = 34 + n_uniq
    comb1 = inp("comb1", [KE, N1])
    comb2 = inp("comb2", [KE, N2])
    emb_block = inp("emb_block", [KE, F])
    rhs1 = inp("rhs1", [F, 208])
    rhs2 = inp("rhs2", [FO, 12])
    rhs2_tmp = inp("rhs2_tmp", [1, 12])
    m_mat = inp("m_mat", [FO, FO])
    mask_pair = inp("mask_pair", [128, 256])
    b1rep = inp("b1rep", [128, FO])
    src1w = inp("src1w", [T1_TILES, 128, W1], mybir.dt.int16)
    dst1w = inp("dst1w", [T1_TILES, 128, W1], mybir.dt.int16)
    d1loc = inp("d1loc", [T1_TILES, 128, nchunk1])
    src2w = inp("src2w", [T2_TILES, 128, W2], mybir.dt.int16)
    dst2w = inp("dst2w", [T2_TILES, 128, W2], mybir.dt.int16)
    d2loc = inp("d2loc", [T2_TILES, 128, nchunk2])

    out_t = nc.dram_tensor("out", [GPC * FUTURE], f32, kind="ExternalOutput")

    t1_dram = nc.dram_tensor("t1_tab", [N1, ROW1], f32, kind="Internal")
    t1d_dram = nc.dram_tensor("t1d_tab", [N1, ROW1D], f32, kind="Internal")
    t2_dram = nc.dram_tensor("t2_tab", [N2, ROW2], f32, kind="Internal")
    y_dram = nc.dram_tensor("y_row", [1, N1], f32, kind="Internal")
    q2_dram = nc.dram_tensor("q2_row", [1, N2], f32, kind="Internal")
    cc_in = nc.dram_tensor("cc_in", [FO, GPC * PAST], f32, kind="Internal")
    cc_out = nc.dram_tensor("cc_out", [N_CORES, FO, GPC * PAST], f32,
                            kind="Internal", addr_space="Shared")

    AF = mybir.ActivationFunctionType
    AL = mybir.AluOpType

    with tile.TileContext(nc) as tc:
        with tc.tile_pool(name="big", bufs=1) as big, \
             tc.tile_pool(name="consts", bufs=1) as consts, \
             tc.tile_pool(name="ps", bufs=2, space="PSUM") as psp, \
             tc.tile_pool(name="ps_acc", bufs=2, space="PSUM") as psa:

            # ---------- constants ----------
            ident = consts.tile([128, 128], f32)
            make_identity(nc, ident)
            iota_row = consts.tile([128, 128], mybir.dt.int32)
            nc.gpsimd.iota(iota_row, pattern=[[1, 128]], base=0, channel_multiplier=0)
            iota_row_f = consts.tile([128, 128], f32)
            nc.vector.tensor_copy(out=iota_row_f, in_=iota_row)
            ones50 = consts.tile([FO, 1], f32)
            nc.vector.memset(ones50, 1.0)
            ones1 = consts.tile([1, 128], f32)
            nc.vector.memset(ones1, 1.0)
            neghalf_col = consts.tile([1, 128], f32)
            nc.vector.memset(neghalf_col, -0.5)
            neghalf_row = consts.tile([1, 256], f32)
            nc.vector.memset(neghalf_row, -0.5)

            rhs1_sb = consts.tile([F, 208], f32)
            nc.sync.dma_start(out=rhs1_sb, in_=rhs1[:, :])
            rhs2_sb = consts.tile([FO, 12], f32)
            nc.sync.dma_start(out=rhs2_sb, in_=rhs2[:, :])
            rhs2t_sb = consts.tile([1, 12], f32)
            nc.sync.dma_start(out=rhs2t_sb, in_=rhs2_tmp[:, :])
            m_sb = consts.tile([FO, FO], f32)
            nc.sync.dma_start(out=m_sb, in_=m_mat[:, :])
            mask_sb = consts.tile([128, 256], f32)
            nc.sync.dma_start(out=mask_sb, in_=mask_pair[:, :])
            b1_sb = consts.tile([128, FO], f32)
            nc.sync.dma_start(out=b1_sb, in_=b1rep[:, :])
            embb_sb = consts.tile([KE, F], f32)
            nc.sync.dma_start(out=embb_sb, in_=emb_block[:, :])

            # ---------- phase A: xT [51, N1], x2T [51, N2] ----------
            x2T = big.tile([F, N2], f32)
            x1T = big.tile([FO, N_CORES, GPC * PAST], f32)
            tmprow = big.tile([1, N2], f32)

            with tc.tile_pool(name="oh", bufs=3) as ohp, \
                 tc.tile_pool(name="wAB", bufs=3) as work, \
                 tc.tile_pool(name="xtp", bufs=1) as xtp:

                def build_xt(dst, comb, n_nodes):
                    for ch in range(n_nodes // 512):
                        sl = slice(ch * 512, (ch + 1) * 512)
                        px = psp.tile([128, 512], f32, tag="pt")
                        cb = ohp.tile([KE, 512], f32, tag="cb")
                        nc.sync.dma_start(out=cb, in_=comb[:, sl])
                        nc.tensor.matmul(px[0:F, :], embb_sb, cb, start=True, stop=True)
                        nc.vector.tensor_copy(out=dst[0:F, sl], in_=px[0:F, :])

                build_xt(x2T, comb2, N2)

                xT = xtp.tile([F, N1], f32)
                build_xt(xT, comb1, N1)
                nc.sync.dma_start(out=y_dram[:, :], in_=xT[FO:F, :])

                # ---------- phase B: T1/T1d tables ----------
                for t in range(N1 // 128):
                    ph = psp.tile([128, 208], f32, tag="pt")
                    nc.tensor.matmul(ph, xT[:, t * 128:(t + 1) * 128], rhs1_sb,
                                     start=True, stop=True)
                    st1 = work.tile([128, ROW1], f32, tag="st1")
                    nc.gpsimd.memset(st1[:, 204:ROW1], 0.0)
                    nc.scalar.copy(out=st1[:, 0:204], in_=ph[:, 0:204])
                    st1d = work.tile([128, ROW1D], f32, tag="st1d")
                    nc.gpsimd.memset(st1d[:, 4:ROW1D], 0.0)
                    nc.vector.tensor_copy(out=st1d[:, 0:4], in_=ph[:, 204:208])
                    nc.sync.dma_start(out=t1_dram[t * 128:(t + 1) * 128, :], in_=st1)
                    nc.sync.dma_start(out=t1d_dram[t * 128:(t + 1) * 128, :], in_=st1d)

            # ---------- phase C: GAT1 sharded by dst tile ----------
            run_c = stage in ("C", "CC", "D", "E", "F")
            run_cc = stage in ("CC", "D", "E", "F")
            run_d = stage in ("D", "E", "F")
            run_e = stage in ("E", "F")
            run_f = stage == "F"
            with tc.tile_pool(name="gb1", bufs=2) as gbp, \
                 tc.tile_pool(name="wC", bufs=3) as work, \
                 tc.tile_pool(name="smC", bufs=2) as small:
                x1Tl = work.tile([FO, T1_TILES, 128], f32, tag="x1Tl")
                nc.gpsimd.memset(x1Tl, 0.0)
                for t in range(T1_TILES) if run_c else []:
                    sidx = small.tile([128, W1], mybir.dt.int16, tag="sidx")
                    nc.sync.dma_start(out=sidx, in_=src1w[t, :, :])
                    didx = small.tile([128, W1], mybir.dt.int16, tag="didx")
                    nc.sync.dma_start(out=didx, in_=dst1w[t, :, :])
                    dl = small.tile([128, nchunk1], f32, tag="dl")
                    nc.sync.dma_start(out=dl, in_=d1loc[t, :, :])

                    gs = gbp.tile([128, nchunk1, ROW1], f32, tag="gs")
                    nc.gpsimd.dma_gather(gs, t1_dram[:, :], sidx, E1P, E1P, ROW1, single_packet=False)
                    gd = gbp.tile([128, nchunk1, ROW1D], f32, tag="gd")
                    nc.gpsimd.dma_gather(gd, t1d_dram[:, :], didx, E1P, E1P, ROW1D, single_packet=False)
                    if csub == "g":
                        nc.vector.tensor_copy(out=x1Tl[0:FO, t, :],
                                              in_=gs[0:FO, 0, 0:128])
                        continue

                    # logits -> ex, in [128, nchunk1, 4]
                    z = small.tile([128, nchunk1, 4], f32, tag="z")
                    nc.vector.tensor_tensor(out=z, in0=gs[:, :, 200:204],
                                            in1=gd[:, :, 0:4], op=AL.add)
                    z2 = small.tile([128, nchunk1, 4], f32, tag="z2")
                    nc.vector.tensor_scalar(out=z2, in0=z, scalar1=0.2, scalar2=None,
                                            op0=AL.mult)
                    nc.vector.tensor_tensor(out=z, in0=z, in1=z2, op=AL.max)
                    ex = small.tile([128, nchunk1, 4], f32, tag="ex")
                    nc.scalar.activation(out=ex, in_=z, func=AF.Exp)
                    if csub == "l":
                        nc.vector.tensor_copy(out=x1Tl[0:FO, t, :],
                                              in_=gs[0:FO, 0, 0:128])
                        continue

                    # msg: gs[:, :, 0:200] *= ex (head-bcast); gs[:, :, 200:204] = ex
                    ex_b = bass.AP(
                        tensor=ex.tensor, offset=ex.offset,
                        ap=[list(ex.ap[0]), [4, nchunk1], [1, 4], [0, FO]])
                    nc.vector.tensor_tensor(out=gs[:, :, 0:200], in0=gs[:, :, 0:200],
                                            in1=ex_b, op=AL.mult)
                    nc.vector.tensor_copy(out=gs[:, :, 200:204], in_=ex)
                    if csub == "m":
                        nc.vector.tensor_copy(out=x1Tl[0:FO, t, :],
                                              in_=gs[0:FO, 0, 0:128])
                        continue

                    po = psa.tile([128, 204], f32, tag="po")
                    for k in range(nchunk1):
                        stk = work.tile([128, 128], f32, tag="stk")
                        nc.vector.tensor_scalar(out=stk, in0=iota_row_f,
                                                scalar1=dl[:, k:k + 1], scalar2=None,
                                                op0=AL.is_equal)
                        nc.tensor.matmul(po, stk, gs[:, k, 0:204],
                                         start=(k == 0), stop=(k == nchunk1 - 1))

                    if csub == "a":
                        nc.vector.tensor_copy(out=x1Tl[0:FO, t, :],
                                              in_=po[0:FO, 0:128])
                        continue
                    den = small.tile([128, 4], f32, tag="den")
                    nc.vector.tensor_scalar(out=den, in0=po[:, 200:204], scalar1=1e-16,
                                            scalar2=None, op0=AL.add)
                    nc.vector.reciprocal(out=den, in_=den)
                    xt1 = work.tile([128, 128], f32, tag="xt1")
                    nc.gpsimd.memset(xt1[:, FO:128], 0.0)
                    nc.vector.tensor_scalar(out=xt1[:, 0:FO], in0=po[:, 0:FO],
                                            scalar1=den[:, 0:1], scalar2=None, op0=AL.mult)
                    tmp50 = work.tile([128, FO], f32, tag="tmp50")
                    for h in range(1, HEADS):
                        nc.vector.tensor_scalar(out=tmp50, in0=po[:, h * FO:(h + 1) * FO],
                                                scalar1=den[:, h:h + 1], scalar2=None,
                                                op0=AL.mult)
                        nc.vector.tensor_tensor(out=xt1[:, 0:FO], in0=xt1[:, 0:FO],
                                                in1=tmp50, op=AL.add)
                    nc.vector.tensor_scalar(out=xt1[:, 0:FO], in0=xt1[:, 0:FO],
                                            scalar1=0.25, scalar2=None, op0=AL.mult)
                    nc.vector.tensor_tensor(out=xt1[:, 0:FO], in0=xt1[:, 0:FO],
                                            in1=b1_sb, op=AL.add)
                    ptr = psp.tile([128, 128], f32, tag="pt")
                    nc.tensor.transpose(ptr, xt1, ident)
                    nc.scalar.copy(out=x1Tl[:, t, :], in_=ptr[0:FO, :])

                nc.sync.dma_start(out=cc_in[:, :],
                                  in_=x1Tl[:, :, :].rearrange("c t p -> c (t p)"))
                if run_cc and not no_cc:
                    nc.gpsimd.collective_compute(
                        "AllGather", AL.bypass,
                        replica_groups=[list(range(N_CORES))],
                        ins=[cc_in[:, :]], outs=[cc_out[:, :, :]],
                    )
                elif run_cc:
                    for rr in range(N_CORES):
                        nc.sync.dma_start(out=cc_out[rr, :, :], in_=cc_in[:, :])
                else:
                    nc.gpsimd.memset(x1T, 0.0)
                if run_cc:
                    nc.sync.dma_start(out=x1T,
                                      in_=cc_out[:, :, :].rearrange("r c n -> c r n"))

            x1Tf = x1T[:, :, :].rearrange("c r n -> c (r n)")  # [50, 8192]

            # ---------- phase D: middle stage (per graph pair) ----------
            with tc.tile_pool(name="wD", bufs=3) as work, \
                 tc.tile_pool(name="smD", bufs=2) as small:
                # q2 row [1, N2] staged through DRAM
                nc.gpsimd.memset(tmprow, 0.0)
                for ch in range(N2 // 512) if run_d else []:
                    sl = slice(ch * 512, (ch + 1) * 512)
                    pb = psp.tile([128, 512], f32, tag="pt")
                    nc.tensor.matmul(pb[0:FO, :], m_sb, x2T[0:FO, sl], start=True, stop=True)
                    xb = work.tile([FO, 512], f32, tag="xb")
                    nc.vector.tensor_tensor(out=xb, in0=x2T[0:FO, sl], in1=pb[0:FO, :],
                                            op=AL.mult)
                    pq = psp.tile([128, 512], f32, tag="pt")
                    nc.tensor.matmul(pq[0:1, :], ones50, xb, start=True, stop=True)
                    q2tmp = work.tile([1, 512], f32, tag="q2tmp")
                    nc.vector.tensor_copy(out=q2tmp, in_=pq[0:1, :])
                    nc.sync.dma_start(out=q2_dram[:, sl], in_=q2tmp)

                for j in range(B // 2) if run_d else []:
                    x1sl = x1Tf[:, j * 256:(j + 1) * 256]
                    pa = psp.tile([128, 256], f32, tag="pt")
                    nc.tensor.matmul(pa[0:FO, :], m_sb, x1sl, start=True, stop=True)
                    a_sb = work.tile([FO, 256], f32, tag="a_sb")
                    nc.scalar.copy(out=a_sb, in_=pa[0:FO, :])
                    xa = work.tile([FO, 256], f32, tag="xa")
                    nc.vector.tensor_tensor(out=xa, in0=x1sl, in1=a_sb, op=AL.mult)
                    pq1 = psp.tile([128, 256], f32, tag="pt")
                    nc.tensor.matmul(pq1[0:1, :], ones50, xa, start=True, stop=True)
                    q1_sb = small.tile([1, 256], f32, tag="q1_sb")
                    nc.vector.tensor_copy(out=q1_sb, in_=pq1[0:1, :])

                    palpha = psa.tile([128, 256], f32, tag="palpha")
                    nc.tensor.matmul(palpha, x2T[0:FO, j * 128:(j + 1) * 128], a_sb,
                                     start=True, stop=False)
                    nc.tensor.matmul(palpha, neghalf_col, q1_sb, start=False, stop=False)
                    q2sl = small.tile([1, 128], f32, tag="q2sl")
                    nc.sync.dma_start(out=q2sl, in_=q2_dram[:, j * 128:(j + 1) * 128])
                    nc.tensor.matmul(palpha, q2sl, neghalf_row, start=False, stop=True)

                    alpha = work.tile([128, 256], f32, tag="alpha")
                    nc.vector.tensor_tensor(out=alpha, in0=palpha, in1=mask_sb, op=AL.add)
                    mx = small.tile([128, 1], f32, tag="mx")
                    nc.vector.tensor_reduce(out=mx, in_=alpha, axis=mybir.AxisListType.X,
                                            op=AL.max)
                    mneg = small.tile([128, 1], f32, tag="mneg")
                    nc.vector.tensor_scalar(out=mneg, in0=mx, scalar1=-1.0, scalar2=None,
                                            op0=AL.mult)
                    ex2 = work.tile([128, 256], f32, tag="ex2")
                    dsum = small.tile([128, 1], f32, tag="dsum")
                    nc.scalar.activation(out=ex2, in_=alpha, func=AF.Exp,
                                         bias=mneg[:, 0:1], accum_out=dsum[:, 0:1])
                    y_sb = small.tile([1, 256], f32, tag="y_sb")
                    nc.sync.dma_start(out=y_sb, in_=y_dram[:, j * 256:(j + 1) * 256])
                    pyb = psp.tile([128, 256], f32, tag="pt")
                    nc.tensor.matmul(pyb, ones1, y_sb, start=True, stop=True)
                    prod = work.tile([128, 256], f32, tag="prod")
                    nc.vector.tensor_tensor(out=prod, in0=ex2, in1=pyb, op=AL.mult)
                    tnum = small.tile([128, 1], f32, tag="tnum")
                    nc.vector.tensor_reduce(out=tnum, in_=prod, axis=mybir.AxisListType.X,
                                            op=AL.add)
                    rden = small.tile([128, 1], f32, tag="rden")
                    nc.vector.reciprocal(out=rden, in_=dsum)
                    tmpv = work.tile([128, 128], f32, tag="tmpv")
                    nc.gpsimd.memset(tmpv[:, 1:128], 0.0)
                    nc.vector.tensor_tensor(out=tmpv[:, 0:1], in0=tnum, in1=rden, op=AL.mult)
                    ptt = psp.tile([128, 128], f32, tag="pt")
                    nc.tensor.transpose(ptt, tmpv, ident)
                    nc.scalar.copy(out=tmprow[0:1, j * 128:(j + 1) * 128], in_=ptt[0:1, :])

            # ---------- phase E: T2 table ----------
            with tc.tile_pool(name="t2p", bufs=1) as t2p:
                stT2 = t2p.tile([128, (N2 // 128) * ROW2], f32)
                nc.gpsimd.memset(stT2, 0.0)
                for t in range(N2 // 128) if run_e else []:
                    p2 = psp.tile([128, 12], f32, tag="pt")
                    nc.tensor.matmul(p2, x2T[0:FO, t * 128:(t + 1) * 128],
                                     rhs2_sb, start=True, stop=False)
                    nc.tensor.matmul(p2, tmprow[0:1, t * 128:(t + 1) * 128],
                                     rhs2t_sb, start=False, stop=True)
                    nc.scalar.copy(out=stT2[:, t * ROW2:t * ROW2 + 12], in_=p2)
                t2_view = bass.AP(
                    tensor=t2_dram, offset=0,
                    ap=[[ROW2, 128], [128 * ROW2, N2 // 128], [1, ROW2]])
                nc.sync.dma_start(out=t2_view, in_=stT2[:, :].rearrange(
                    "p (t r) -> p t r", r=ROW2))

            # ---------- phase F: GAT2 sharded ----------
            with tc.tile_pool(name="gb2", bufs=2) as gbp, \
                 tc.tile_pool(name="wF", bufs=3) as work, \
                 tc.tile_pool(name="smF", bufs=2) as small:
                zout = consts.tile([128, 1], f32)
                nc.vector.memset(zout, 0.0)
                for t in range(T2_TILES) if run_f else []:
                    sidx = small.tile([128, W2], mybir.dt.int16, tag="sidx2")
                    nc.sync.dma_start(out=sidx, in_=src2w[t, :, :])
                    didx = small.tile([128, W2], mybir.dt.int16, tag="didx2")
                    nc.sync.dma_start(out=didx, in_=dst2w[t, :, :])
                    dl = small.tile([128, nchunk2], f32, tag="dl2")
                    nc.sync.dma_start(out=dl, in_=d2loc[t, :, :])

                    gs = gbp.tile([128, nchunk2, ROW2], f32, tag="gs2")
                    nc.gpsimd.dma_gather(gs, t2_dram[:, :], sidx, E2P, E2P, ROW2, single_packet=False)
                    gd = gbp.tile([128, nchunk2, ROW2], f32, tag="gd2")
                    nc.gpsimd.dma_gather(gd, t2_dram[:, :], didx, E2P, E2P, ROW2, single_packet=False)

                    z = small.tile([128, nchunk2, 4], f32, tag="z_2")
                    nc.vector.tensor_tensor(out=z, in0=gs[:, :, 4:8],
                                            in1=gd[:, :, 8:12], op=AL.add)
                    z2 = small.tile([128, nchunk2, 4], f32, tag="z2_2")
                    nc.vector.tensor_scalar(out=z2, in0=z, scalar1=0.2, scalar2=None,
                                            op0=AL.mult)
                    nc.vector.tensor_tensor(out=z, in0=z, in1=z2, op=AL.max)
                    ex = small.tile([128, nchunk2, 4], f32, tag="ex_2")
                    nc.scalar.activation(out=ex, in_=z, func=AF.Exp)
                    nc.vector.tensor_tensor(out=gs[:, :, 0:4], in0=gs[:, :, 0:4],
                                            in1=ex, op=AL.mult)
                    nc.vector.tensor_copy(out=gs[:, :, 4:8], in_=ex)

                    po = psa.tile([128, 8], f32, tag="po2")
                    for k in range(nchunk2):
                        stk = work.tile([128, 128], f32, tag="stk2")
                        nc.vector.tensor_scalar(out=stk, in0=iota_row_f,
                                                scalar1=dl[:, k:k + 1], scalar2=None,
                                                op0=AL.is_equal)
                        nc.tensor.matmul(po, stk, gs[:, k, 0:8],
                                         start=(k == 0), stop=(k == nchunk2 - 1))

                    den = small.tile([128, 4], f32, tag="den2")
                    nc.vector.tensor_scalar(out=den, in0=po[:, 4:8], scalar1=1e-16,
                                            scalar2=None, op0=AL.add)
                    nc.vector.reciprocal(out=den, in_=den)
                    prod = small.tile([128, 4], f32, tag="prod2")
                    nc.vector.tensor_tensor(out=prod, in0=po[:, 0:4], in1=den, op=AL.mult)
                    osum = small.tile([128, 1], f32, tag="osum")
                    nc.vector.tensor_reduce(out=osum, in_=prod, axis=mybir.AxisListType.X,
                                            op=AL.add)
                    ofin = small.tile([128, 1], f32, tag="ofin")
                    nc.vector.tensor_scalar(out=ofin, in0=osum, scalar1=0.25,
                                            scalar2=b2, op0=AL.mult, op1=AL.add)
                    nc.sync.dma_start(out=out_t[t * 128:(t + 1) * 128], in_=ofin)
                if not run_f:
                    for t in range(T2_TILES):
                        nc.sync.dma_start(out=out_t[t * 128:(t + 1) * 128], in_=zout)

    nc.compile()
    return nc


last_result = None


def kernel(**inputs):
    global last_result
    nchunk1, nchunk2, n_uniq, b2, per_core = _prep_inputs(**inputs)
    key = (nchunk1, nchunk2, n_uniq, round(b2, 10), os.environ.get("K_STAGE", "F"), os.environ.get("K_NOCC", "0"), os.environ.get("K_CSUB", "full"))
    if key not in _cache:
        _cache[key] = _build(nchunk1, nchunk2, n_uniq, b2)
    nc = _cache[key]
    r = run_bass_kernel_spmd(nc, per_core, core_ids=list(range(N_CORES)))
    last_result = r
    out = np.concatenate([r.results[c]["out"] for c in range(N_CORES)])
    return out.reshape(B, FUTURE).astype(np.float32)


# revision 21
# speedup vs baseline: 1.1972x; 1.1972x over previous
"""Trainium2 Bass kernel for nn_GAT_44487271252524.

GAT -> per-graph pairwise attention -> GAT, data-parallel over the 64 graphs
(8 graphs per NeuronCore): the edge message-passing is sharded by destination
node, everything cheap is replicated, and one AllGather moves the first GAT's
output (x1^T) between the sharded and replicated stages.

Message passing: per-edge dma_gather of bf16 table rows [h|a_src|a_dst] by
src id; per-dst-tile one-hot scatter matrices (host-precomputed, bf16) feed
the tensor engine for both the a_dst per-edge lookup (each tile's chunk 0 is
its 128 self-loops, so row d of the gather buffer holds a_dst[d]) and the
alpha-weighted segment sum, accumulated in f32 PSUM.
"""
import os
import numpy as np

import concourse.bass as bass
import concourse.bacc as bacc
import concourse.mybir as mybir
import concourse.tile as tile
from concourse.bass_utils import run_bass_kernel_spmd
from concourse.masks import make_identity

N_CORES = 8
B = 64
PAST = 128
FUTURE = 64
HEADS = 4
F = 51          # input feature dim
FO = F - 1      # GAT1 output dim (50)
N1 = B * PAST   # 8192 past nodes
N2 = B * FUTURE  # 4096 future nodes
GPC = B // N_CORES             # graphs per core (8)
T1_TILES = GPC * PAST // 128   # GAT1 dst tiles per core (8)
T2_TILES = GPC * FUTURE // 128  # GAT2 dst tiles per core (4)
ROW1 = 256      # bf16 per T1 row (512B): [h(200) | asrc(4) | adst(4) | pad]
ROW2 = 64       # f32 per T2 row (256B): [h2(4) | asrc2(4) | adst2(4) | pad]
NEG = -1.0e30

_cache = {}


def _wrap_idx(idx):
    """int16 indices -> dma_gather layout [128, n/16]: idx i at [i%16, i//16],
    replicated across the 8 Q7 core groups."""
    n = idx.shape[0]
    out = np.zeros((128, n // 16), dtype=np.int16)
    w = idx.reshape(n // 16, 16).T
    for g in range(8):
        out[g * 16:(g + 1) * 16, :] = w
    return out


def _edge_prep(src, dst, n_dst_per_core, tile_count, out_dtype=None):
    """Split non-self-loop edges by (core, 128-dst tile); each tile's chunk 0
    is its 128 self-loops in dst order. Returns per-core wrapped src index
    arrays plus scatter/gather one-hot masks (bf16)."""
    import ml_dtypes
    order = np.argsort(dst, kind="stable")
    src = src[order]
    dst = dst[order]
    n_total = n_dst_per_core * N_CORES
    tkey = dst // 128   # global tile id; tiles are contiguous per core
    counts = np.bincount(tkey, minlength=n_total // 128)
    nchunk = 1 + int(np.ceil(counts.max() / 128))   # +1 for self-loop chunk
    epad = nchunk * 128
    starts = np.searchsorted(tkey, np.arange(n_total // 128))
    ends = np.searchsorted(tkey, np.arange(n_total // 128), side="right")
    srcw = np.zeros((N_CORES, tile_count, 128, epad // 16), dtype=np.int16)
    stm = np.zeros((N_CORES, tile_count, nchunk, 128, 128), dtype=np.float32)
    sm = np.zeros((N_CORES, tile_count, nchunk, 128, 128), dtype=np.float32)
    for c in range(N_CORES):
        for t in range(tile_count):
            g = c * tile_count + t
            base = g * 128
            a, b_ = starts[g], ends[g]
            k = b_ - a
            s_full = np.zeros(epad, dtype=np.int64)
            dloc = np.full(epad, -1, dtype=np.int64)
            # chunk 0: self loops in dst order
            s_full[0:128] = base + np.arange(128)
            dloc[0:128] = np.arange(128)
            s_full[128:128 + k] = src[a:b_]
            dloc[128:128 + k] = dst[a:b_] - base
            srcw[c, t] = _wrap_idx(s_full.astype(np.int16))
            dl = dloc.reshape(nchunk, 128)
            for ck in range(nchunk):
                oh = (dl[ck][:, None] ==
                      np.arange(128)[None, :]).astype(np.float32)
                stm[c, t, ck] = oh          # [e, d] for the scatter lhsT
                sm[c, t, ck] = oh.T         # [d, e] for the a_dst lookup lhsT
    if out_dtype is None:
        out_dtype = ml_dtypes.bfloat16
    return nchunk, srcw, stm.astype(out_dtype), sm.astype(out_dtype)


def _prep_inputs(cat1, num1, cat2, num2, e1, e2, A, emb0, emb1, emb2,
                 g1_lin, g1_asrc, g1_adst, g1_b, g2_lin, g2_asrc, g2_adst,
                 g2_b, W):
    f32 = np.float32
    cat1 = np.asarray(cat1).astype(np.int64)
    cat2 = np.asarray(cat2).astype(np.int64)
    e1 = np.asarray(e1).astype(np.int64)
    e2 = np.asarray(e2).astype(np.int64)

    # compacted emb2 + combined one-hot/passthrough rhs for the xT build
    uniq = np.unique(np.concatenate([cat1[:, 2], cat2[:, 2]]))
    assert uniq.shape[0] <= 94, "too many distinct emb2 indices"
    remap = np.zeros(int(uniq.max()) + 1, dtype=np.int64)
    remap[uniq] = np.arange(uniq.shape[0])
    emb2c = np.asarray(emb2, f32)[uniq]          # [U, 24]
    U = uniq.shape[0]

    def onehot(vals, depth):
        oh = np.zeros((depth, vals.shape[0]), dtype=f32)
        oh[vals, np.arange(vals.shape[0])] = 1.0
        return oh

    num1T = np.ascontiguousarray(np.asarray(num1, f32).T)   # [3, N1]
    num2T = np.ascontiguousarray(np.asarray(num2, f32).T)   # [3, N2]
    comb1 = np.concatenate([onehot(cat1[:, 0], 24), onehot(cat1[:, 1], 7),
                            onehot(remap[cat1[:, 2]], U), num1T])
    comb2 = np.concatenate([onehot(cat2[:, 0], 24), onehot(cat2[:, 1], 7),
                            onehot(remap[cat2[:, 2]], U), num2T])
    emb_block = np.zeros((34 + U, F), dtype=f32)
    emb_block[0:24, 0:16] = np.asarray(emb0, f32)
    emb_block[24:31, 16:24] = np.asarray(emb1, f32)
    emb_block[31:31 + U, 24:48] = emb2c
    emb_block[31 + U:34 + U, 48:51] = np.eye(3, dtype=f32)

    g1_lin = np.asarray(g1_lin, f32)
    g1_asrc = np.asarray(g1_asrc, f32)
    g1_adst = np.asarray(g1_adst, f32)
    w1_asrc = np.stack([g1_lin[:, h * FO:(h + 1) * FO] @ g1_asrc[h]
                        for h in range(HEADS)], axis=1)     # [51, 4]
    w1_adst = np.stack([g1_lin[:, h * FO:(h + 1) * FO] @ g1_adst[h]
                        for h in range(HEADS)], axis=1)
    rhs1 = np.concatenate([g1_lin, w1_asrc, w1_adst], axis=1)  # [51, 208]

    g2_lin = np.asarray(g2_lin, f32)
    w2_asrc = g2_lin * np.asarray(g2_asrc, f32)[:, 0][None, :]  # [51, 4]
    w2_adst = g2_lin * np.asarray(g2_adst, f32)[:, 0][None, :]
    rhs2 = np.concatenate([g2_lin, w2_asrc, w2_adst], axis=1)   # [51, 12]

    W = np.asarray(W, f32)
    M = W @ W.T
    M = (M + M.T).astype(f32)                                   # [50, 50]

    maskA = np.where(np.asarray(A)[:PAST, PAST:].T == 0, f32(NEG), f32(0.0))
    mask_pair = np.full((128, 256), f32(NEG), dtype=f32)
    mask_pair[0:64, 0:128] = maskA
    mask_pair[64:128, 128:256] = maskA

    b1rep = np.tile(np.asarray(g1_b, f32)[None, :], (128, 1))   # [128, 50]
    b2 = float(np.asarray(g2_b, f32)[0])

    nchunk1, srcw1, stm1, sm1 = _edge_prep(e1[0], e1[1], N1 // N_CORES,
                                           T1_TILES)
    nchunk2, srcw2, stm2, sm2 = _edge_prep(e2[0], e2[1], N2 // N_CORES,
                                           T2_TILES, out_dtype=np.float32)

    shared = dict(
        comb1=comb1, comb2=comb2, emb_block=emb_block,
        rhs1=rhs1, rhs2=rhs2[0:FO].copy(), rhs2_tmp=rhs2[FO:F].copy(),
        m_mat=M, mask_pair=mask_pair, b1rep=b1rep,
    )
    per_core = []
    for c in range(N_CORES):
        d = dict(shared)
        d["src1w"] = srcw1[c]
        d["stm1"] = stm1[c]
        d["sm1"] = sm1[c]
        d["src2w"] = srcw2[c]
        d["stm2"] = stm2[c]
        d["sm2"] = sm2[c]
        per_core.append(d)
    return nchunk1, nchunk2, U, b2, per_core


def _build(nchunk1, nchunk2, n_uniq, b2):
    f32 = mybir.dt.float32
    bf16 = mybir.dt.bfloat16
    nc = bacc.Bacc("TRN2", target_bir_lowering=False, num_devices=N_CORES,
                   num_swdge_queues=2)
    E1P = nchunk1 * 128
    E2P = nchunk2 * 128
    KE = 34 + n_uniq

    def inp(name, shape, dtype=f32):
        return nc.dram_tensor(name, shape, dtype, kind="ExternalInput")

    comb1 = inp("comb1", [KE, N1])
    comb2 = inp("comb2", [KE, N2])
    emb_block = inp("emb_block", [KE, F])
    rhs1 = inp("rhs1", [F, 208])
    rhs2 = inp("rhs2", [FO, 12])
    rhs2_tmp = inp("rhs2_tmp", [1, 12])
    m_mat = inp("m_mat", [FO, FO])
    mask_pair = inp("mask_pair", [128, 256])
    b1rep = inp("b1rep", [128, FO])
    src1w = inp("src1w", [T1_TILES, 128, E1P // 16], mybir.dt.int16)
    stm1 = inp("stm1", [T1_TILES, nchunk1, 128, 128], bf16)
    sm1 = inp("sm1", [T1_TILES, nchunk1, 128, 128], bf16)
    src2w = inp("src2w", [T2_TILES, 128, E2P // 16], mybir.dt.int16)
    stm2 = inp("stm2", [T2_TILES, nchunk2, 128, 128])
    sm2 = inp("sm2", [T2_TILES, nchunk2, 128, 128])

    out_t = nc.dram_tensor("out", [GPC * FUTURE], f32, kind="ExternalOutput")

    t1_dram = nc.dram_tensor("t1_tab", [N1, ROW1], bf16, kind="Internal")
    t2_dram = nc.dram_tensor("t2_tab", [N2, ROW2], f32, kind="Internal")
    y_dram = nc.dram_tensor("y_row", [1, N1], f32, kind="Internal")
    q2_dram = nc.dram_tensor("q2_row", [1, N2], f32, kind="Internal")
    cc_in = nc.dram_tensor("cc_in", [FO, GPC * PAST], f32, kind="Internal")
    cc_out = nc.dram_tensor("cc_out", [N_CORES, FO, GPC * PAST], f32,
                            kind="Internal", addr_space="Shared")

    AF = mybir.ActivationFunctionType
    AL = mybir.AluOpType

    with tile.TileContext(nc) as tc:
        with tc.tile_pool(name="big", bufs=1) as big, \
             tc.tile_pool(name="consts", bufs=1) as consts, \
             tc.tile_pool(name="ps", bufs=2, space="PSUM") as psp, \
             tc.tile_pool(name="ps_sm", bufs=2, space="PSUM") as pss, \
             tc.tile_pool(name="ps_acc", bufs=2, space="PSUM") as psa:

            ident = consts.tile([128, 128], f32)
            make_identity(nc, ident)
            ones50 = consts.tile([FO, 1], f32)
            nc.vector.memset(ones50, 1.0)
            ones1 = consts.tile([1, 128], f32)
            nc.vector.memset(ones1, 1.0)
            neghalf_col = consts.tile([1, 128], f32)
            nc.vector.memset(neghalf_col, -0.5)
            neghalf_row = consts.tile([1, 256], f32)
            nc.vector.memset(neghalf_row, -0.5)

            rhs1_sb = consts.tile([F, 208], f32)
            nc.sync.dma_start(out=rhs1_sb, in_=rhs1[:, :])
            rhs2_sb = consts.tile([FO, 12], f32)
            nc.sync.dma_start(out=rhs2_sb, in_=rhs2[:, :])
            rhs2t_sb = consts.tile([1, 12], f32)
            nc.sync.dma_start(out=rhs2t_sb, in_=rhs2_tmp[:, :])
            m_sb = consts.tile([FO, FO], f32)
            nc.sync.dma_start(out=m_sb, in_=m_mat[:, :])
            mask_sb = consts.tile([128, 256], f32)
            nc.sync.dma_start(out=mask_sb, in_=mask_pair[:, :])
            b1_sb = consts.tile([128, FO], f32)
            nc.sync.dma_start(out=b1_sb, in_=b1rep[:, :])
            embb_sb = consts.tile([KE, F], f32)
            nc.sync.dma_start(out=embb_sb, in_=emb_block[:, :])

            x2T = big.tile([F, N2], f32)
            x1T = big.tile([FO, N_CORES, GPC * PAST], f32)
            tmprow = big.tile([1, N2], f32)

            # ---------- phase A: xT / x2T; phase B: T1 table ----------
            with tc.tile_pool(name="oh", bufs=3) as ohp, \
                 tc.tile_pool(name="wAB", bufs=3) as work, \
                 tc.tile_pool(name="xtp", bufs=1) as xtp:

                def build_xt(dst, comb, n_nodes):
                    for ch in range(n_nodes // 512):
                        sl = slice(ch * 512, (ch + 1) * 512)
                        px = psp.tile([128, 512], f32, tag="pt")
                        cb = ohp.tile([KE, 512], f32, tag="cb")
                        nc.sync.dma_start(out=cb, in_=comb[:, sl])
                        nc.tensor.matmul(px[0:F, :], embb_sb, cb,
                                         start=True, stop=True)
                        nc.vector.tensor_copy(out=dst[0:F, sl], in_=px[0:F, :])

                build_xt(x2T, comb2, N2)

                xT = xtp.tile([F, N1], f32)
                build_xt(xT, comb1, N1)
                nc.sync.dma_start(out=y_dram[:, :], in_=xT[FO:F, :])

                for t in range(N1 // 128):
                    ph = psp.tile([128, 208], f32, tag="pt")
                    nc.tensor.matmul(ph, xT[:, t * 128:(t + 1) * 128], rhs1_sb,
                                     start=True, stop=True)
                    st1 = work.tile([128, ROW1], bf16, tag="st1")
                    nc.gpsimd.memset(st1[:, 208:ROW1], 0.0)
                    nc.scalar.copy(out=st1[:, 0:208], in_=ph[:, 0:208])
                    nc.sync.dma_start(out=t1_dram[t * 128:(t + 1) * 128, :],
                                      in_=st1)

            # ---------- phase C: GAT1 sharded by dst tile ----------
            with tc.tile_pool(name="gb1", bufs=2) as gbp, \
                 tc.tile_pool(name="msk", bufs=2) as mskp, \
                 tc.tile_pool(name="wC", bufs=3) as work, \
                 tc.tile_pool(name="smC", bufs=2) as small:
                x1Tl = work.tile([FO, T1_TILES, 128], f32, tag="x1Tl")
                for t in range(T1_TILES):
                    sidx = small.tile([128, E1P // 16], mybir.dt.int16,
                                      tag="sidx")
                    nc.sync.dma_start(out=sidx, in_=src1w[t, :, :])
                    stma = mskp.tile([128, nchunk1, 128], bf16, tag="stma")
                    nc.sync.dma_start(
                        out=stma,
                        in_=stm1[t, :, :, :].rearrange("k e d -> e k d"))
                    sma = mskp.tile([128, nchunk1, 128], bf16, tag="sma")
                    nc.sync.dma_start(
                        out=sma,
                        in_=sm1[t, :, :, :].rearrange("k d e -> d k e"))

                    gs = gbp.tile([128, nchunk1, ROW1], bf16, tag="gs")
                    nc.gpsimd.dma_gather(gs, t1_dram[:, :], sidx, E1P, E1P,
                                         ROW1, single_packet=False,
                                         queue_num=t % 2)

                    # a_dst of this dst tile = self-loop rows (chunk 0)
                    adst_t = small.tile([128, 4], bf16, tag="adst_t")
                    nc.vector.tensor_copy(out=adst_t, in_=gs[:, 0, 204:208])

                    # per-chunk a_dst lookup + logits
                    zbuf = small.tile([128, nchunk1, 4], f32, tag="zbuf")
                    for k in range(nchunk1):
                        aps = pss.tile([128, 4], f32, tag="ps_s")
                        nc.tensor.matmul(aps, sma[:, k, :], adst_t,
                                         start=True, stop=True)
                        nc.vector.tensor_tensor(out=zbuf[:, k, :],
                                                in0=gs[:, k, 200:204],
                                                in1=aps, op=AL.add)
                    z2 = small.tile([128, nchunk1, 4], f32, tag="z2")
                    nc.vector.tensor_scalar(out=z2, in0=zbuf, scalar1=0.2,
                                            scalar2=None, op0=AL.mult)
                    nc.vector.tensor_tensor(out=zbuf, in0=zbuf, in1=z2,
                                            op=AL.max)
                    ex = small.tile([128, nchunk1, 4], f32, tag="ex")
                    nc.scalar.activation(out=ex, in_=zbuf, func=AF.Exp)

                    # msg: gs[:,:,0:200] *= ex (head-bcast); cols 200:204 = ex
                    ex_b = bass.AP(
                        tensor=ex.tensor, offset=ex.offset,
                        ap=[list(ex.ap[0]), [4, nchunk1], [1, 4], [0, FO]])
                    nc.vector.tensor_tensor(out=gs[:, :, 0:200],
                                            in0=gs[:, :, 0:200],
                                            in1=ex_b, op=AL.mult)
                    nc.vector.tensor_copy(out=gs[:, :, 200:204], in_=ex)

                    po = psa.tile([128, 204], f32, tag="acc")
                    for k in range(nchunk1):
                        nc.tensor.matmul(po, stma[:, k, :], gs[:, k, 0:204],
                                         start=(k == 0),
                                         stop=(k == nchunk1 - 1))

                    den = small.tile([128, 4], f32, tag="den")
                    nc.vector.tensor_scalar(out=den, in0=po[:, 200:204],
                                            scalar1=1e-16, scalar2=None,
                                            op0=AL.add)
                    nc.vector.reciprocal(out=den, in_=den)
                    xt1 = work.tile([128, 128], f32, tag="xt1")
                    nc.gpsimd.memset(xt1[:, FO:128], 0.0)
                    nc.vector.tensor_scalar(out=xt1[:, 0:FO], in0=po[:, 0:FO],
                                            scalar1=den[:, 0:1], scalar2=None,
                                            op0=AL.mult)
                    tmp50 = work.tile([128, FO], f32, tag="tmp50")
                    for h in range(1, HEADS):
                        nc.vector.tensor_scalar(
                            out=tmp50, in0=po[:, h * FO:(h + 1) * FO],
                            scalar1=den[:, h:h + 1], scalar2=None, op0=AL.mult)
                        nc.vector.tensor_tensor(out=xt1[:, 0:FO],
                                                in0=xt1[:, 0:FO],
                                                in1=tmp50, op=AL.add)
                    nc.vector.tensor_scalar(out=xt1[:, 0:FO], in0=xt1[:, 0:FO],
                                            scalar1=0.25, scalar2=None,
                                            op0=AL.mult)
                    nc.vector.tensor_tensor(out=xt1[:, 0:FO], in0=xt1[:, 0:FO],
                                            in1=b1_sb, op=AL.add)
                    ptr = psp.tile([128, 128], f32, tag="pt")
                    nc.tensor.transpose(ptr, xt1, ident)
                    nc.scalar.copy(out=x1Tl[:, t, :], in_=ptr[0:FO, :])

                nc.sync.dma_start(
                    out=cc_in[:, :],
                    in_=x1Tl[:, :, :].rearrange("c t p -> c (t p)"))
                nc.gpsimd.collective_compute(
                    "AllGather", AL.bypass,
                    replica_groups=[list(range(N_CORES))],
                    ins=[cc_in[:, :]], outs=[cc_out[:, :, :]],
                )
                nc.sync.dma_start(
                    out=x1T, in_=cc_out[:, :, :].rearrange("r c n -> c r n"))

            x1Tf = x1T[:, :, :].rearrange("c r n -> c (r n)")  # [50, 8192]

            # ---------- phase D: middle stage (per graph pair) ----------
            with tc.tile_pool(name="wD", bufs=3) as work, \
                 tc.tile_pool(name="smD", bufs=2) as small:
                nc.gpsimd.memset(tmprow, 0.0)
                for ch in range(N2 // 512):
                    sl = slice(ch * 512, (ch + 1) * 512)
                    pb = psp.tile([128, 512], f32, tag="pt")
                    nc.tensor.matmul(pb[0:FO, :], m_sb, x2T[0:FO, sl],
                                     start=True, stop=True)
                    xb = work.tile([FO, 512], f32, tag="xb")
                    nc.vector.tensor_tensor(out=xb, in0=x2T[0:FO, sl],
                                            in1=pb[0:FO, :], op=AL.mult)
                    pq = psp.tile([128, 512], f32, tag="pt")
                    nc.tensor.matmul(pq[0:1, :], ones50, xb, start=True,
                                     stop=True)
                    q2tmp = work.tile([1, 512], f32, tag="q2tmp")
                    nc.vector.tensor_copy(out=q2tmp, in_=pq[0:1, :])
                    nc.sync.dma_start(out=q2_dram[:, sl], in_=q2tmp)

                for j in range(B // 2):
                    x1sl = x1Tf[:, j * 256:(j + 1) * 256]
                    pa = psp.tile([128, 256], f32, tag="pt")
                    nc.tensor.matmul(pa[0:FO, :], m_sb, x1sl, start=True,
                                     stop=True)
                    a_sb = work.tile([FO, 256], f32, tag="a_sb")
                    nc.scalar.copy(out=a_sb, in_=pa[0:FO, :])
                    xa = work.tile([FO, 256], f32, tag="xa")
                    nc.vector.tensor_tensor(out=xa, in0=x1sl, in1=a_sb,
                                            op=AL.mult)
                    pq1 = psp.tile([128, 256], f32, tag="pt")
                    nc.tensor.matmul(pq1[0:1, :], ones50, xa, start=True,
                                     stop=True)
                    q1_sb = small.tile([1, 256], f32, tag="q1_sb")
                    nc.vector.tensor_copy(out=q1_sb, in_=pq1[0:1, :])

                    palpha = psa.tile([128, 256], f32, tag="acc")
                    nc.tensor.matmul(palpha, x2T[0:FO, j * 128:(j + 1) * 128],
                                     a_sb, start=True, stop=False)
                    nc.tensor.matmul(palpha, neghalf_col, q1_sb, start=False,
                                     stop=False)
                    q2sl = small.tile([1, 128], f32, tag="q2sl")
                    nc.sync.dma_start(out=q2sl,
                                      in_=q2_dram[:, j * 128:(j + 1) * 128])
                    nc.tensor.matmul(palpha, q2sl, neghalf_row, start=False,
                                     stop=True)

                    alpha = work.tile([128, 256], f32, tag="alpha")
                    nc.vector.tensor_tensor(out=alpha, in0=palpha, in1=mask_sb,
                                            op=AL.add)
                    mx = small.tile([128, 1], f32, tag="mx")
                    nc.vector.tensor_reduce(out=mx, in_=alpha,
                                            axis=mybir.AxisListType.X,
                                            op=AL.max)
                    mneg = small.tile([128, 1], f32, tag="mneg")
                    nc.vector.tensor_scalar(out=mneg, in0=mx, scalar1=-1.0,
                                            scalar2=None, op0=AL.mult)
                    ex2 = work.tile([128, 256], f32, tag="ex2")
                    dsum = small.tile([128, 1], f32, tag="dsum")
                    nc.scalar.activation(out=ex2, in_=alpha, func=AF.Exp,
                                         bias=mneg[:, 0:1],
                                         accum_out=dsum[:, 0:1])
                    y_sb = small.tile([1, 256], f32, tag="y_sb")
                    nc.sync.dma_start(out=y_sb,
                                      in_=y_dram[:, j * 256:(j + 1) * 256])
                    pyb = psp.tile([128, 256], f32, tag="pt")
                    nc.tensor.matmul(pyb, ones1, y_sb, start=True, stop=True)
                    prod = work.tile([128, 256], f32, tag="prod")
                    nc.vector.tensor_tensor(out=prod, in0=ex2, in1=pyb,
                                            op=AL.mult)
                    tnum = small.tile([128, 1], f32, tag="tnum")
                    nc.vector.tensor_reduce(out=tnum, in_=prod,
                                            axis=mybir.AxisListType.X,
                                            op=AL.add)
                    rden = small.tile([128, 1], f32, tag="rden")
                    nc.vector.reciprocal(out=rden, in_=dsum)
                    tmpv = work.tile([128, 128], f32, tag="tmpv")
                    nc.gpsimd.memset(tmpv[:, 1:128], 0.0)
                    nc.vector.tensor_tensor(out=tmpv[:, 0:1], in0=tnum,
                                            in1=rden, op=AL.mult)
                    ptt = psp.tile([128, 128], f32, tag="pt")
                    nc.tensor.transpose(ptt, tmpv, ident)
                    nc.scalar.copy(out=tmprow[0:1, j * 128:(j + 1) * 128],
                                   in_=ptt[0:1, :])

            # ---------- phase E: T2 table ----------
            with tc.tile_pool(name="t2p", bufs=1) as t2p:
                stT2 = t2p.tile([128, (N2 // 128) * ROW2], f32)
                nc.gpsimd.memset(stT2, 0.0)
                for t in range(N2 // 128):
                    p2 = psp.tile([128, 12], f32, tag="pt")
                    nc.tensor.matmul(p2, x2T[0:FO, t * 128:(t + 1) * 128],
                                     rhs2_sb, start=True, stop=False)
                    nc.tensor.matmul(p2, tmprow[0:1, t * 128:(t + 1) * 128],
                                     rhs2t_sb, start=False, stop=True)
                    nc.scalar.copy(out=stT2[:, t * ROW2:t * ROW2 + 12], in_=p2)
                t2_view = bass.AP(
                    tensor=t2_dram, offset=0,
                    ap=[[ROW2, 128], [128 * ROW2, N2 // 128], [1, ROW2]])
                nc.sync.dma_start(out=t2_view, in_=stT2[:, :].rearrange(
                    "p (t r) -> p t r", r=ROW2))

            # ---------- phase F: GAT2 sharded ----------
            with tc.tile_pool(name="gb2", bufs=2) as gbp, \
                 tc.tile_pool(name="msk2", bufs=2) as mskp, \
                 tc.tile_pool(name="smF", bufs=2) as small:
                for t in range(T2_TILES):
                    sidx = small.tile([128, E2P // 16], mybir.dt.int16,
                                      tag="sidx2")
                    nc.sync.dma_start(out=sidx, in_=src2w[t, :, :])
                    stma = mskp.tile([128, nchunk2, 128], f32, tag="stma2")
                    nc.sync.dma_start(
                        out=stma,
                        in_=stm2[t, :, :, :].rearrange("k e d -> e k d"))
                    sma = mskp.tile([128, nchunk2, 128], f32, tag="sma2")
                    nc.sync.dma_start(
                        out=sma,
                        in_=sm2[t, :, :, :].rearrange("k d e -> d k e"))

                    gs = gbp.tile([128, nchunk2, ROW2], f32, tag="gs2")
                    nc.gpsimd.dma_gather(gs, t2_dram[:, :], sidx, E2P, E2P,
                                         ROW2, single_packet=False,
                                         queue_num=t % 2)

                    adst_t = small.tile([128, 4], f32, tag="adst2_t")
                    nc.vector.tensor_copy(out=adst_t, in_=gs[:, 0, 8:12])

                    zbuf = small.tile([128, nchunk2, 4], f32, tag="zbuf2")
                    for k in range(nchunk2):
                        aps = pss.tile([128, 4], f32, tag="ps_s")
                        nc.tensor.matmul(aps, sma[:, k, :], adst_t,
                                         start=True, stop=True)
                        nc.vector.tensor_tensor(out=zbuf[:, k, :],
                                                in0=gs[:, k, 4:8],
                                                in1=aps, op=AL.add)
                    z2 = small.tile([128, nchunk2, 4], f32, tag="z2_2")
                    nc.vector.tensor_scalar(out=z2, in0=zbuf, scalar1=0.2,
                                            scalar2=None, op0=AL.mult)
                    nc.vector.tensor_tensor(out=zbuf, in0=zbuf, in1=z2,
                                            op=AL.max)
                    ex = small.tile([128, nchunk2, 4], f32, tag="ex_2")
                    nc.scalar.activation(out=ex, in_=zbuf, func=AF.Exp)
                    nc.vector.tensor_tensor(out=gs[:, :, 0:4],
                                            in0=gs[:, :, 0:4],
                                            in1=ex, op=AL.mult)
                    nc.vector.tensor_copy(out=gs[:, :, 4:8], in_=ex)

                    po = psa.tile([128, 8], f32, tag="acc")
                    for k in range(nchunk2):
                        nc.tensor.matmul(po, stma[:, k, :], gs[:, k, 0:8],
                                         start=(k == 0),
                                         stop=(k == nchunk2 - 1))

                    den = small.tile([128, 4], f32, tag="den2")
                    nc.vector.tensor_scalar(out=den, in0=po[:, 4:8],
                                            scalar1=1e-16, scalar2=None,
                                            op0=AL.add)
                    nc.vector.reciprocal(out=den, in_=den)
                    prod = small.tile([128, 4], f32, tag="prod2")
                    nc.vector.tensor_tensor(out=prod, in0=po[:, 0:4], in1=den,
                                            op=AL.mult)
                    osum = small.tile([128, 1], f32, tag="osum")
                    nc.vector.tensor_reduce(out=osum, in_=prod,
                                            axis=mybir.AxisListType.X,
                                            op=AL.add)
                    ofin = small.tile([128, 1], f32, tag="ofin")
                    nc.vector.tensor_scalar(out=ofin, in0=osum, scalar1=0.25,
                                            scalar2=b2, op0=AL.mult,
                                            op1=AL.add)
                    nc.sync.dma_start(out=out_t[t * 128:(t + 1) * 128],
                                      in_=ofin)

    nc.compile()
    return nc


last_result = None


def kernel(**inputs):
    global last_result
    nchunk1, nchunk2, n_uniq, b2, per_core = _prep_inputs(**inputs)
    key = (nchunk1, nchunk2, n_uniq, round(b2, 10))
    if key not in _cache:
        _cache[key] = _build(nchunk1, nchunk2, n_uniq, b2)
    nc = _cache[key]
    r = run_bass_kernel_spmd(nc, per_core, core_ids=list(range(N_CORES)))
    last_result = r
    out = np.concatenate([r.results[c]["out"] for c in range(N_CORES)])
    return out.reshape(B, FUTURE).astype(np.float32)
